# revision 2
# baseline (speedup 1.0000x reference)
"""Trainium2 Bass kernel for nn_DCTLinearFactored.

Math: reference computes
    coeff[b,i,j] = basis[i] @ x2d[b] @ basis[j]        (2D DCT)
    result[b]    = sum_ij coeff[b,i,j] w_h[i] w_v[j]
    out[b]       = sigmoid(result[b] + bias)

The rank-1 weight collapses the whole thing to a bilinear form:
    result[b] = u^T x2d[b] v,   u = basis^T w_h,  v = basis^T w_v
i.e. one streaming pass over x. The kernel is HBM-bandwidth bound, so the
host re-encodes x in 3 bytes/element (weight-independent):
    x ~= xhi (fp16) + 2^-10 * xl8 (fp8 e4m3 of the scaled fp16 residual)

Device strategy (per core, 32 batch rows = 16 pairs):
  - Chunk-major layout: for batch b, partition p carries x2d row k=128c+p of
    k-chunk c; free axis is [bb, c, l]. Per-pair DMAs (1 MB hi fp16 +
    0.5 MB lo fp8) are ALL pre-issued up front on both HWDGE rings
    (sync+scalar, alternating) into fully-resident SBUF buffers, so the 16
    SDMA engines stream back-to-back with no issue-side gaps.
  - Per b: 4 hi matmuls (stationary [uhi|ulo] chunk, M=2, N=512) accumulate
    u^T x2d[b] into a (2,512) psum; lo stream likewise into (1,512) psum
    (optionally fp8 DoubleRow: chunk-pairs packed along K, halving mms).
  - VectorE: multiply psum rows by v (lo: v/1024) and reduce over l into
    per-b columns of R_hi (2,32) / R_lo (1,32).
  - One fold matmul pair (ones stationary) sums the 3 rows per b into a
    (1,32) psum; ScalarE applies sigmoid(+bias); one 128 B DMA out.
"""

import os

import numpy as np

N = 512
BATCH = 256
NCORES = 8
BPC = BATCH // NCORES          # batch rows per core = 32
PAIRS = BPC // 2               # 16
LO_SCALE = 1024.0              # xl8 holds (x - xhi) * LO_SCALE
K_DR = int(os.environ.get("K_DR", "1"))     # fp8 DoubleRow for lo stream

_CACHE = {}


def _dct_basis_np(n):
    u = np.arange(n)
    cu = np.where(u == 0, np.sqrt(1.0 / n), np.sqrt(2.0 / n))
    cos = np.cos((2.0 * u[:, None] + 1.0) * u[None, :] * np.pi / (2.0 * n))
    return (cu * cos).T.astype(np.float32)  # (n, n), row k = freq-k basis


def _build_nc():
    import concourse.bacc as bacc
    import concourse.bass as bass
    import concourse.mybir as mybir
    import concourse.tile as tile

    f32 = mybir.dt.float32
    f16 = mybir.dt.float16
    f8 = mybir.dt.float8e4
    nc = bacc.Bacc(
        "TRN2", target_bir_lowering=False, debug=False, num_devices=NCORES
    )
    xph_h = nc.dram_tensor("xph", [PAIRS, 128, 4096], f16, kind="ExternalInput")
    xpl_h = nc.dram_tensor(
        "xpl", [PAIRS, 128, 2, 2, 2, 512], f8, kind="ExternalInput"
    )
    um_h = nc.dram_tensor("um", [128, 4, 2], f16, kind="ExternalInput")
    # uq2[p, ko, cp*? ] : ko-major with 16-wide padded cp so the DoubleRow
    # stationary ko-step is 16 bytes; col cp used, cols 4.. zero.
    uq_h = nc.dram_tensor("uq", [128, 2, 16], f8, kind="ExternalInput")
    cst_h = nc.dram_tensor("cst", [2, 1027], f32, kind="ExternalInput")
    out_h = nc.dram_tensor("out", [1, BPC], f32, kind="ExternalOutput")

    with tile.TileContext(nc) as tc:
        with (
            tc.tile_pool(name="const", bufs=1) as cpool,
            tc.tile_pool(name="xp", bufs=PAIRS) as xpool,
            tc.tile_pool(name="sc", bufs=2) as spool,
            tc.tile_pool(name="ps", bufs=4, space=bass.MemorySpace.PSUM) as pspool,
        ):
            cst_t = cpool.tile([2, 1027], f32)
            nc.scalar.dma_start(cst_t[:], cst_h[:])
            um_t = cpool.tile([128, 4, 2], f16)
            nc.scalar.dma_start(um_t[:], um_h[:])
            uq_t = cpool.tile([128, 2, 16], f8)
            nc.scalar.dma_start(uq_t[:], uq_h[:])
            r_hi = cpool.tile([2, BPC], f32)
            r_lo = cpool.tile([1, BPC], f32)
            o_t = cpool.tile([1, BPC], f32)

            v_hi = cst_t[0:2, 0:512]
            v_lo = cst_t[0:1, 512:1024]
            f_hi = cst_t[0:2, 1024:1025]
            f_lo = cst_t[0:1, 1025:1026]
            b_t = cst_t[0:1, 1026:1027]

            # pre-issue every x DMA on alternating HWDGE rings
            xh_tiles = []
            xl_tiles = []
            for pr in range(PAIRS):
                xh_p = xpool.tile([128, 4096], f16, tag="xh", name=f"xh{pr}")
                xl_p = xpool.tile(
                    [128, 2, 2, 2, 512], f8, tag="xl", name=f"xl{pr}"
                )
                ring_a = nc.sync if pr % 2 == 0 else nc.scalar
                ring_b = nc.scalar if pr % 2 == 0 else nc.sync
                ring_a.dma_start(xh_p[:], xph_h[pr])
                ring_b.dma_start(xl_p[:], xpl_h[pr])
                xh_tiles.append(xh_p)
                xl_tiles.append(xl_p)

            for pr in range(PAIRS):
                xh_p = xh_tiles[pr]
                xl_p = xl_tiles[pr]
                ph = [
                    pspool.tile([2, 512], f32, tag="hi", name=f"ph{pr}_{bb}")
                    for bb in range(2)
                ]
                pl = [
                    pspool.tile([1, 512], f32, tag="lo", name=f"pl{pr}_{bb}")
                    for bb in range(2)
                ]
                # hi stream: c-outer so both halves of the pair share the
                # stationary [uhi|ulo] chunk
                for c in range(4):
                    for bb in range(2):
                        nc.tensor.matmul(
                            ph[bb][:],
                            um_t[:, c],
                            xh_p[:, 2048 * bb + 512 * c : 2048 * bb + 512 * (c + 1)],
                            start=(c == 0),
                            stop=(c == 3),
                        )
                if K_DR:
                    for cp in range(2):
                        for bb in range(2):
                            nc.tensor.matmul(
                                pl[bb][:],
                                uq_t[:, :, cp : cp + 1],
                                xl_p[:, bb, cp],
                                start=(cp == 0),
                                stop=(cp == 1),
                                perf_mode=mybir.MatmulPerfMode.DoubleRow,
                            )
                else:
                    for c in range(4):
                        for bb in range(2):
                            nc.tensor.matmul(
                                pl[bb][:],
                                uq_t[:, c % 2, (c // 2) : (c // 2) + 1],
                                xl_p[:, bb, c // 2, c % 2],
                                start=(c == 0),
                                stop=(c == 3),
                            )
                for bb in range(2):
                    b = 2 * pr + bb
                    sch = spool.tile([2, 512], f32, tag="sch", name=f"sch{b}")
                    nc.vector.tensor_tensor(
                        out=sch[:], in0=ph[bb][:], in1=v_hi,
                        op=mybir.AluOpType.mult,
                    )
                    nc.vector.tensor_reduce(
                        out=r_hi[:, b : b + 1], in_=sch[:],
                        axis=mybir.AxisListType.X, op=mybir.AluOpType.add,
                    )
                    scl = spool.tile([1, 512], f32, tag="scl", name=f"scl{b}")
                    nc.vector.tensor_tensor(
                        out=scl[:], in0=pl[bb][:], in1=v_lo,
                        op=mybir.AluOpType.mult,
                    )
                    nc.vector.tensor_reduce(
                        out=r_lo[:, b : b + 1], in_=scl[:],
                        axis=mybir.AxisListType.X, op=mybir.AluOpType.add,
                    )

            fold = pspool.tile([1, BPC], f32, tag="lo", name="fold")
            nc.tensor.matmul(fold[:], f_hi, r_hi[:], start=True, stop=False)
            nc.tensor.matmul(fold[:], f_lo, r_lo[:], start=False, stop=True)
            nc.scalar.activation(
                o_t[:],
                fold[:],
                mybir.ActivationFunctionType.Sigmoid,
                bias=b_t,
            )
            nc.sync.dma_start(out_h[:], o_t[:])
    nc.compile()
    return nc


def _get_nc():
    if "nc" not in _CACHE:
        _CACHE["nc"] = _build_nc()
    return _CACHE["nc"]


def _host_prep(x, w_horizontal, w_vertical, bias):
    import ml_dtypes

    f8 = ml_dtypes.float8_e4m3
    basis = _dct_basis_np(N).astype(np.float64)  # (n, n) row k = freq k
    u = (np.asarray(w_horizontal, np.float64) @ basis).astype(np.float32)
    v = (np.asarray(w_vertical, np.float64) @ basis).astype(np.float32)
    uhi = u.astype(np.float16).astype(np.float32)
    ulo = (u - uhi).astype(np.float16)
    uq = u.astype(f8)

    um = np.zeros((128, 4, 2), np.float16)
    uq2 = np.zeros((128, 2, 16), f8)
    p = np.arange(128)
    for c in range(4):
        um[p, c, 0] = uhi.astype(np.float16)[128 * c + p]
        um[p, c, 1] = ulo[128 * c + p]
    for cp in range(2):
        for ko in range(2):
            uq2[p, ko, cp] = uq[128 * (2 * cp + ko) + p]

    cst = np.zeros((2, 1027), np.float32)
    cst[:, 0:512] = v[None, :]
    cst[0, 512:1024] = v / LO_SCALE
    cst[:, 1024] = 1.0
    cst[0, 1025] = 1.0
    cst[0, 1026] = float(np.asarray(bias).reshape(-1)[0])

    x = np.ascontiguousarray(np.asarray(x, np.float32))
    xhi16 = x.astype(np.float16)
    xlo8 = ((x - xhi16.astype(np.float32)) * LO_SCALE).astype(f8)
    # [core, pair, bb, c, p, l] -> [core, pair, p, bb, c, l]
    H = xhi16.reshape(NCORES, PAIRS, 2, 4, 128, 512).transpose(0, 1, 4, 2, 3, 5)
    L = xlo8.reshape(NCORES, PAIRS, 2, 4, 128, 512).transpose(0, 1, 4, 2, 3, 5)
    H = np.ascontiguousarray(H)
    L = np.ascontiguousarray(L)
    in_maps = []
    for i in range(NCORES):
        in_maps.append(
            {
                "xph": H[i].reshape(PAIRS, 128, 4096),
                # [pair, p, bb, c, l] with c=2*cp+ko -> [pair,p,bb,cp,ko,l]
                "xpl": L[i].reshape(PAIRS, 128, 2, 2, 2, 512),
                "um": um,
                "uq": uq2,
                "cst": cst,
            }
        )
    return in_maps


def _run(x, w_horizontal, w_vertical, bias, trace=False):
    from concourse.bass_utils import run_bass_kernel_spmd

    nc = _get_nc()
    in_maps = _host_prep(x, w_horizontal, w_vertical, bias)
    res = run_bass_kernel_spmd(
        nc, in_maps, core_ids=list(range(NCORES)), trace=trace
    )
    parts = [
        np.asarray(res.results[i]["out"]).reshape(BPC) for i in range(NCORES)
    ]
    full = np.concatenate(parts).astype(np.float32)[:, None]
    return full, res


def kernel(x, w_horizontal, w_vertical, bias):
    out, _ = _run(x, w_horizontal, w_vertical, bias, trace=False)
    return out


# revision 7
# speedup vs baseline: 1.1975x; 1.1975x over previous
"""Trainium2 Bass kernel for nn_DCTLinearFactored.

Math: reference computes
    coeff[b,i,j] = basis[i] @ x2d[b] @ basis[j]        (2D DCT)
    result[b]    = sum_ij coeff[b,i,j] w_h[i] w_v[j]
    out[b]       = sigmoid(result[b] + bias)

The rank-1 weight collapses the whole thing to a bilinear form:
    result[b] = u^T x2d[b] v,   u = basis^T w_h,  v = basis^T w_v
i.e. one streaming pass over x. The kernel is HBM-bandwidth bound, so the
host re-encodes x in 3 bytes/element (weight-independent):
    x ~= xhi (fp16) + 2^-10 * xl8 (fp8 e4m3 of the scaled fp16 residual)

Device strategy (per core, 32 batch rows = 16 pairs):
  - Chunk-major layout: for batch b, partition p carries x2d row k=128c+p of
    k-chunk c; free axis is [bb, c, l]. Per-pair DMAs (1 MB hi fp16 +
    0.5 MB lo fp8) are ALL pre-issued up front on both HWDGE rings
    (sync+scalar, alternating) into fully-resident SBUF buffers, so the 16
    SDMA engines stream back-to-back with no issue-side gaps.
  - Per pair ONE psum bank, 3 col-group slots (base 96 = quadrant 3 is
    unusable on TRN2; DoubleRow fp8 is incompatible with col tiling so the
    lo block sits at base 0): lo BOTH rows 0-1 via zero-masked M=2
    stationaries ([uq|0] for b0, [0|uq] for b1) accumulating into one block
    (fp8 DoubleRow packs chunk-pairs along K); hi b0 rows 32-33 (stationary
    [uhi|ulo] chunk, M=2, N=512, 4 chunk-accumulated matmuls); hi b1 rows
    64-65. Matmuls are interleaved hi/lo so consecutive ones hit different
    col groups and LDWEIGHTS pulls ahead.
  - VectorE stage 2 per pair: ONE (66,512) multiply by VV (v on hi rows,
    v/1024 on lo rows, 0 elsewhere) + ONE reduce into column pr of R.
  - One fold matmul (3-row selector stationary) -> (2,16) logits; ScalarE
    sigmoid(+bias); one 128 B DMA out.
"""

import os

import numpy as np

N = 512
BATCH = 256
NCORES = 8
BPC = BATCH // NCORES          # batch rows per core = 32
PAIRS = BPC // 2               # 16
LO_SCALE = 1024.0              # xl8 holds (x - xhi) * LO_SCALE
K_DR = int(os.environ.get("K_DR", "1"))     # fp8 DoubleRow for lo stream

_CACHE = {}


def _dct_basis_np(n):
    u = np.arange(n)
    cu = np.where(u == 0, np.sqrt(1.0 / n), np.sqrt(2.0 / n))
    cos = np.cos((2.0 * u[:, None] + 1.0) * u[None, :] * np.pi / (2.0 * n))
    return (cu * cos).T.astype(np.float32)  # (n, n), row k = freq-k basis


def _build_nc():
    import concourse.bacc as bacc
    import concourse.bass as bass
    import concourse.mybir as mybir
    import concourse.tile as tile

    f32 = mybir.dt.float32
    f16 = mybir.dt.float16
    f8 = mybir.dt.float8e4
    nc = bacc.Bacc(
        "TRN2", target_bir_lowering=False, debug=False, num_devices=NCORES
    )
    xph_h = nc.dram_tensor("xph", [PAIRS, 128, 4096], f16, kind="ExternalInput")
    xpl_h = nc.dram_tensor(
        "xpl", [PAIRS, 128, 2, 2, 2, 512], f8, kind="ExternalInput"
    )
    um_h = nc.dram_tensor("um", [128, 4, 2], f16, kind="ExternalInput")
    # uqz[p, cp, z, ko, m(pad16)]: DoubleRow stationary [uq|0] / [0|uq];
    # m padded to 16 so the dual-fp8 ldweights ko-step is 16 bytes
    uqz_h = nc.dram_tensor("uqz", [128, 2, 2, 2, 16], f8, kind="ExternalInput")
    # uqm[p, c, z, m]: non-DR masked stationary
    uqm_h = nc.dram_tensor("uqm", [128, 4, 2, 2], f8, kind="ExternalInput")
    vv_h = nc.dram_tensor("vv", [66, 512], f32, kind="ExternalInput")
    ms_h = nc.dram_tensor("ms", [66, 3], f32, kind="ExternalInput")
    out_h = nc.dram_tensor("out", [2, PAIRS], f32, kind="ExternalOutput")

    with tile.TileContext(nc) as tc:
        with (
            tc.tile_pool(name="const", bufs=1) as cpool,
            tc.tile_pool(name="xp", bufs=PAIRS) as xpool,
            tc.tile_pool(name="sc", bufs=2) as spool,
            tc.tile_pool(name="ps", bufs=1, space=bass.MemorySpace.PSUM) as pspool,
        ):
            um_t = cpool.tile([128, 4, 2], f16)
            nc.scalar.dma_start(um_t[:], um_h[:])
            uqz_t = cpool.tile([128, 2, 2, 2, 16], f8)
            nc.scalar.dma_start(uqz_t[:], uqz_h[:])
            uqm_t = cpool.tile([128, 4, 2, 2], f8)
            nc.scalar.dma_start(uqm_t[:], uqm_h[:])
            vv_t = cpool.tile([66, 512], f32)
            nc.scalar.dma_start(vv_t[:], vv_h[:])
            ms_t = cpool.tile([66, 3], f32)
            nc.scalar.dma_start(ms_t[:], ms_h[:])
            r_t = cpool.tile([66, PAIRS], f32)
            o_t = cpool.tile([2, PAIRS], f32)

            f_t = ms_t[:, 0:2]
            b_t = ms_t[0:2, 2:3]

            # 8 persistent psum banks, one per in-flight pair
            pbs = [
                pspool.tile([66, 512], f32, tag=f"pb{i}", name=f"pb{i}")
                for i in range(8)
            ]
            for i in range(8):
                nc.vector.memset(pbs[i][:], 0)

            # pre-issue every x DMA on alternating HWDGE rings
            xh_tiles = []
            xl_tiles = []
            for pr in range(PAIRS):
                xh_p = xpool.tile([128, 4096], f16, tag="xh", name=f"xh{pr}")
                xl_p = xpool.tile(
                    [128, 2, 2, 2, 512], f8, tag="xl", name=f"xl{pr}"
                )
                ring_a = nc.sync if pr % 2 == 0 else nc.scalar
                ring_b = nc.scalar if pr % 2 == 0 else nc.sync
                ring_a.dma_start(xh_p[:], xph_h[pr])
                ring_b.dma_start(xl_p[:], xpl_h[pr])
                xh_tiles.append(xh_p)
                xl_tiles.append(xl_p)

            for pr in range(PAIRS):
                xh_p = xh_tiles[pr]
                xl_p = xl_tiles[pr]
                pb = pbs[pr % 8]

                def hi_mm(c, bb):
                    nc.tensor.matmul(
                        pb[32 + 32 * bb : 34 + 32 * bb, :],
                        um_t[:, c],
                        xh_p[:, 2048 * bb + 512 * c : 2048 * bb + 512 * (c + 1)],
                        start=(c == 0),
                        stop=(c == 3),
                    )

                if K_DR:
                    def lo_mm(i, bb):  # i = chunk-pair cp
                        nc.tensor.matmul(
                            pb[0:2, :],
                            uqz_t[:, i, bb, :, 0:2],
                            xl_p[:, bb, i],
                            start=(i == 0 and bb == 0),
                            stop=(i == 1 and bb == 1),
                            perf_mode=mybir.MatmulPerfMode.DoubleRow,
                        )
                    lo_sched = [(0, 0), (0, 1), (1, 0), (1, 1)]
                else:
                    def lo_mm(i, bb):  # i = chunk c
                        nc.tensor.matmul(
                            pb[0:2, :],
                            uqm_t[:, i, bb],
                            xl_p[:, bb, i // 2, i % 2],
                            start=(i == 0 and bb == 0),
                            stop=(i == 3 and bb == 1),
                        )
                    lo_sched = [(i, bb) for i in range(4) for bb in range(2)]

                # interleave hi/lo so consecutive matmuls alternate col grps
                nlo = len(lo_sched)
                li = 0
                for c in range(4):
                    hi_mm(c, 0)
                    hi_mm(c, 1)
                    while li * 4 < nlo * (c + 1):
                        lo_mm(*lo_sched[li])
                        li += 1

                sc = spool.tile([66, 512], f32, tag="sc", name=f"sc{pr}")
                nc.vector.tensor_tensor(
                    out=sc[:], in0=pb[:], in1=vv_t[:], op=mybir.AluOpType.mult
                )
                nc.vector.tensor_reduce(
                    out=r_t[:, pr : pr + 1], in_=sc[:],
                    axis=mybir.AxisListType.X, op=mybir.AluOpType.add,
                )

            fold = pbs[0][0:2, 0:PAIRS]
            nc.tensor.matmul(fold, f_t, r_t[:], start=True, stop=True)
            nc.scalar.activation(
                o_t[:],
                fold,
                mybir.ActivationFunctionType.Sigmoid,
                bias=b_t,
            )
            nc.sync.dma_start(out_h[:], o_t[:])
    nc.compile()
    return nc


def _get_nc():
    if "nc" not in _CACHE:
        _CACHE["nc"] = _build_nc()
    return _CACHE["nc"]


def _host_prep(x, w_horizontal, w_vertical, bias):
    import ml_dtypes

    f8 = ml_dtypes.float8_e4m3
    basis = _dct_basis_np(N).astype(np.float64)  # (n, n) row k = freq k
    u = (np.asarray(w_horizontal, np.float64) @ basis).astype(np.float32)
    v = (np.asarray(w_vertical, np.float64) @ basis).astype(np.float32)
    uhi = u.astype(np.float16).astype(np.float32)
    ulo = (u - uhi).astype(np.float16)
    uq = u.astype(f8)

    um = np.zeros((128, 4, 2), np.float16)
    uqz = np.zeros((128, 2, 2, 2, 16), f8)
    uqm = np.zeros((128, 4, 2, 2), f8)
    p = np.arange(128)
    for c in range(4):
        um[p, c, 0] = uhi.astype(np.float16)[128 * c + p]
        um[p, c, 1] = ulo[128 * c + p]
        for z in range(2):
            uqm[p, c, z, z] = uq[128 * c + p]
    for cp in range(2):
        for z in range(2):
            for ko in range(2):
                uqz[p, cp, z, ko, z] = uq[128 * (2 * cp + ko) + p]

    # VV: v/1024 on lo rows {0,1}, v on hi rows {32,33,64,65}, else 0
    vv = np.zeros((66, 512), np.float32)
    vv[[32, 33, 64, 65], :] = v[None, :]
    vv[[0, 1], :] = (v / LO_SCALE)[None, :]
    # ms: cols 0-1 = fold selectors, col 2 = bias
    ms = np.zeros((66, 3), np.float32)
    ms[[32, 33, 0], 0] = 1.0   # b0 rows (hi b0 + lo b0)
    ms[[64, 65, 1], 1] = 1.0   # b1 rows (hi b1 + lo b1)
    ms[0:2, 2] = float(np.asarray(bias).reshape(-1)[0])

    x = np.ascontiguousarray(np.asarray(x, np.float32))
    xhi16 = x.astype(np.float16)
    xlo8 = ((x - xhi16.astype(np.float32)) * LO_SCALE).astype(f8)
    # [core, pair, bb, c, p, l] -> [core, pair, p, bb, c, l]
    H = xhi16.reshape(NCORES, PAIRS, 2, 4, 128, 512).transpose(0, 1, 4, 2, 3, 5)
    L = xlo8.reshape(NCORES, PAIRS, 2, 4, 128, 512).transpose(0, 1, 4, 2, 3, 5)
    H = np.ascontiguousarray(H)
    L = np.ascontiguousarray(L)
    in_maps = []
    for i in range(NCORES):
        in_maps.append(
            {
                "xph": H[i].reshape(PAIRS, 128, 4096),
                # [pair, p, bb, c, l] with c=2*cp+ko -> [pair,p,bb,cp,ko,l]
                "xpl": L[i].reshape(PAIRS, 128, 2, 2, 2, 512),
                "um": um,
                "uqz": uqz,
                "uqm": uqm,
                "vv": vv,
                "ms": ms,
            }
        )
    return in_maps


def _run(x, w_horizontal, w_vertical, bias, trace=False):
    from concourse.bass_utils import run_bass_kernel_spmd

    nc = _get_nc()
    in_maps = _host_prep(x, w_horizontal, w_vertical, bias)
    res = run_bass_kernel_spmd(
        nc, in_maps, core_ids=list(range(NCORES)), trace=trace
    )
    # out[bb, pr] = batch row 2*pr + bb of this core's shard
    parts = [
        np.asarray(res.results[i]["out"]).T.reshape(BPC) for i in range(NCORES)
    ]
    full = np.concatenate(parts).astype(np.float32)[:, None]
    return full, res


def kernel(x, w_horizontal, w_vertical, bias):
    out, _ = _run(x, w_horizontal, w_vertical, bias, trace=False)
    return out
